# revision 10
# baseline (speedup 1.0000x reference)
"""Trainium2 Bass kernel for the DiffRenderer problem.

Math refactor (validated against the jax reference):
  The renderer's per-point MLP input collapses to
      a[b, pix, d, h] = depth[b, d] * g[b, pix, h] + e[b, h]
  with
      g[b] = Q @ V_b,  V_b = M_b^T @ W1[:3],  M_b = s_obj * R_obj
      e[b] = c_b @ W1[:3] + z_shape[b] @ W1[3:] + b1
      depth[b, d] = zs[d] * s_inv[b] + bb_depth[b]
  Layer 1 + relu:  h' = max(depth*g, -e)   (== relu(a) - e)
  Layer 2:         s  = W2 . h' + (W2 . e + b2)   (bias folded into tanh)
  sdf = tanh(s); then the zero-crossing depth extraction.

Sharding: 8 cores = 4 batches x 2 pixel-halves (2048 pixels/core, 64 depths).

v2 schedule (vs baseline):
  - all small constants packed into one [128,69] DMA; qs|vb packed [3,2176];
    input DMAs split sync queue (qsvb, consts) / gpsimd queue (w2w, zm1)
  - 6 junk warm-up matmuls on a memset tile while waiting for the qsvb DMA
    (HAM un-throttles the PE clock ~3.4us after activity starts)
  - d-loop h' split across THREE engines: DVE cols 0:1216 (mult+max),
    GPSIMD 1216:1536 (mult+max), ACT 1536:2048 (relu w/ scale+bias);
    one shared h' tile so layer-2 runs as 4 aligned 512-col matmuls
  - tail: per-chunk copy(ACT)->transpose(PE)->tanh(ACT, psum src)->
    pos(DVE); zc/m1/m3 on GPSIMD, m2+reduces+finals on DVE; occ DMA
    issued before the s1/s2 reductions
Precision: float32r (FP22 in the PE) for g and layer-2, fp32 elsewhere;
  bf16 was measured to flip sdf signs in the randn-weight regime and is
  deliberately NOT used.
"""

import os
import sys

import numpy as np

for _p in ("/opt/trn_rl_repo", "/root/.axon_site/_ro/trn_rl_repo"):
    if os.path.isdir(_p) and _p not in sys.path:
        sys.path.append(_p)

from contextlib import ExitStack

from concourse import bacc, bass, masks, mybir, tile
from concourse.bass_utils import run_bass_kernel_spmd

F32 = mybir.dt.float32
F32R = mybir.dt.float32r
ALU = mybir.AluOpType
ACTF = mybir.ActivationFunctionType

IMG = 64
D = 64
HID = 128
BS = 4
NCORES = 8
PIX = IMG * IMG          # 4096 pixels per batch
PPC = PIX // 2           # 2048 pixels per core
NT = PPC // 128          # 16 pixel tiles per core
K63 = 63                 # depth pairs per tile

# engine split of the 2048 h' columns per depth. The boundary does NOT
# need 128-alignment: the boundary tile (tile 10) gets a host-packed
# per-partition mixed tanh bias. Balanced from measured engine rates
# (DVE 179+0.59/col, ACT 149+1.21/col).
SPLIT = 1360             # DVE: 0:1360 (~981 ns/depth) max-form
                         # ACT: 1360:2048 (~982 ns/depth) relu-form
NCH = 4                  # psum chunks of 512

# consts packing: [128, 70] = depth(0:64) | nege(64) | epos(65) | bias0(66)
#                 | bias1(67) | lam(68) | bias_mix(69)
C_DEPTH, C_NEGE, C_EPOS, C_B0, C_B1, C_LAM, C_BMIX = 0, 64, 65, 66, 67, 68, 69

_PROGRAM = None


def build_program():
    nc = bacc.Bacc(None, target_bir_lowering=False)
    qsvb = nc.declare_dram_parameter("qsvb", [3, PPC + HID], F32R, isOutput=False)
    w2w = nc.declare_dram_parameter("w2w", [HID, 127], F32R, isOutput=False)
    consts = nc.declare_dram_parameter("consts", [128, 70], F32, isOutput=False)
    zm1 = nc.declare_dram_parameter("zm1", [128, NT * K63], F32, isOutput=False)
    dp_o = nc.declare_dram_parameter("dp", [128, NT], F32, isOutput=True)
    occ_o = nc.declare_dram_parameter("occ", [128, NT], F32, isOutput=True)

    with tile.TileContext(nc) as tc, ExitStack() as ctx:
        const = ctx.enter_context(tc.tile_pool(name="const", bufs=1))
        gpool = ctx.enter_context(tc.tile_pool(name="g", bufs=1))
        hpool = ctx.enter_context(tc.tile_pool(name="hp", bufs=3))
        spool = ctx.enter_context(tc.tile_pool(name="s", bufs=1))
        post = ctx.enter_context(tc.tile_pool(name="post", bufs=1))
        pss = ctx.enter_context(tc.tile_pool(name="pss", bufs=1, space="PSUM"))
        pst = ctx.enter_context(tc.tile_pool(name="pst", bufs=1, space="PSUM"))

        # ---- input DMAs: critical ones on the sync queue, rest on gpsimd
        t_qsvb = const.tile([3, PPC + HID], F32R, name="t_qsvb", tag="t_qsvb")
        nc.sync.dma_start(t_qsvb[:], qsvb[:])
        t_consts = const.tile([128, 70], F32, name="t_consts", tag="t_consts")
        nc.sync.dma_start(t_consts[:], consts[:])
        w2win = const.tile([HID, 127], F32R, name="t_w2w", tag="t_w2w")
        nc.gpsimd.dma_start(w2win[:], w2w[:])
        t_zm1 = const.tile([128, NT * K63], F32, name="t_zm1", tag="t_zm1")
        nc.gpsimd.dma_start(t_zm1[:], zm1[:])

        t_depth = t_consts[:, C_DEPTH:C_DEPTH + 64]
        t_nege = t_consts[:, C_NEGE:C_NEGE + 1]
        t_epos = t_consts[:, C_EPOS:C_EPOS + 1]
        t_bias0 = t_consts[:, C_B0:C_B0 + 1]
        t_bias1 = t_consts[:, C_B1:C_B1 + 1]
        t_lam = t_consts[:, C_LAM:C_LAM + 1]
        t_bmix = t_consts[:, C_BMIX:C_BMIX + 1]

        ident = const.tile([64, 64], F32)
        masks.make_identity(nc, ident[:])

        # ---- PE warm-up: junk matmuls on the identity tile while the qsvb
        # DMA is in flight (start=True on the real g matmul discards them).
        pg_w = [
            pss.tile([64, 64], F32, name=f"pgw{k}", tag=f"pss{k}")
            for k in range(2)
        ]
        for j in range(12):
            nc.tensor.matmul(
                pg_w[j % 2][:], ident[:], ident[:],
                start=True, stop=True, skip_group_check=True,
            )

        # ---- g = Q @ V  (float32r matmul; one-time) ----
        g_sb = gpool.tile([HID, PPC], F32)
        t_vb = t_qsvb[:, PPC:PPC + HID]
        for k in range(NCH):
            pg = pss.tile([HID, 512], F32, name=f"pg{k}", tag=f"pss{k}")
            nc.tensor.matmul(
                pg[:], t_vb, t_qsvb[:, k * 512:(k + 1) * 512],
                start=True, stop=True, skip_group_check=True,
            )
            dst = g_sb[:, k * 512:(k + 1) * 512]
            if k % 2 == 0:
                nc.vector.tensor_copy(dst, pg[:])
            else:
                nc.scalar.copy(dst, pg[:])

        # ---- main d-loop ----
        ps_chunks = [
            pss.tile([D, 512], F32, name=f"ps_chunk{k}", tag=f"pss{k}")
            for k in range(NCH)
        ]
        for d in range(D):
            hp = hpool.tile([HID, PPC], F32R, tag="hp")
            dd = t_depth[:, d:d + 1]
            if d == 0:
                # split along the g-copy chunks so work starts before the
                # last g copy lands
                for o0, o1 in ((0, 512), (512, 1024), (1024, SPLIT)):
                    nc.vector.tensor_scalar(
                        hp[:, o0:o1], g_sb[:, o0:o1], dd, t_nege,
                        op0=ALU.mult, op1=ALU.max,
                    )
            else:
                nc.vector.tensor_scalar(
                    hp[:, 0:SPLIT], g_sb[:, 0:SPLIT], dd, t_nege,
                    op0=ALU.mult, op1=ALU.max,
                )
            nc.scalar.activation(
                hp[:, SPLIT:PPC], g_sb[:, SPLIT:PPC], ACTF.Relu,
                bias=t_epos, scale=dd,
            )
            w2slice = w2win[:, 63 - d:127 - d]
            for k in range(NCH):
                nc.tensor.matmul(
                    ps_chunks[k][:], w2slice, hp[:, k * 512:(k + 1) * 512],
                    start=(d == 0), stop=(d == D - 1),
                    skip_group_check=True,
                )

        # ---- tail ----
        s_sb = spool.tile([D, PPC], F32)
        sdf = post.tile([128, NT * D], F32)
        pos = post.tile([128, NT * D], F32)
        zc = post.tile([128, NT * K63], F32)
        m1 = post.tile([128, NT * K63], F32)
        m2 = post.tile([128, NT * K63], F32)
        m3 = post.tile([128, NT * K63], F32)
        d1r = post.tile([128, NT], F32)
        s1 = post.tile([128, NT], F32)
        s2 = post.tile([128, NT], F32)
        sdf3 = sdf[:].rearrange("p (t d) -> p t d", d=D)
        pos3 = pos[:].rearrange("p (t d) -> p t d", d=D)
        zc3 = zc[:].rearrange("p (t k) -> p t k", k=K63)
        m1_3 = m1[:].rearrange("p (t k) -> p t k", k=K63)
        m2_3 = m2[:].rearrange("p (t k) -> p t k", k=K63)
        m3_3 = m3[:].rearrange("p (t k) -> p t k", k=K63)
        zm1_3 = t_zm1[:].rearrange("p (t k) -> p t k", k=K63)

        # per chunk: copy -> 4 transposes -> tanh (bias0 for the max-form
        # chunks 0..2, bias1 for the ACT relu-form chunk 3) -> pos
        for k in range(NCH):
            nc.scalar.copy(s_sb[:, k * 512:(k + 1) * 512], ps_chunks[k][:])
            pt = pst.tile([128, 4 * D], F32, name=f"pt{k}", tag="pt", bufs=2)
            for j in range(4):
                i = 4 * k + j
                nc.tensor.transpose(
                    pt[:, j * D:(j + 1) * D],
                    s_sb[:, i * 128:(i + 1) * 128], ident[:])
            if k == 2:
                # chunk 2 holds the DVE/ACT boundary tile (tile 10):
                # tiles 8-9 bias0, tile 10 per-partition mixed, tile 11 bias1
                for lo, hi, b_ap in ((0, 2, t_bias0), (2, 3, t_bmix),
                                     (3, 4, t_bias1)):
                    nc.scalar.activation(
                        sdf[:, (8 + lo) * D:(8 + hi) * D],
                        pt[:, lo * D:hi * D], ACTF.Tanh,
                        bias=b_ap, scale=1.0,
                    )
            else:
                b_ap = t_bias0 if k < 3 else t_bias1
                nc.scalar.activation(
                    sdf[:, 4 * k * D:4 * (k + 1) * D], pt[:], ACTF.Tanh,
                    bias=b_ap, scale=1.0,
                )
            nc.vector.tensor_scalar(
                pos[:, 4 * k * D:4 * (k + 1) * D],
                sdf[:, 4 * k * D:4 * (k + 1) * D], 0.0, None, op0=ALU.is_gt)

        # crossing logic in two tile groups; zc on DVE (GPSIMD lacks the
        # scalar_tensor_tensor opcode), the three mask-mults on GPSIMD
        for t0, t1 in ((0, 8), (8, 16)):
            ts_ = slice(t0, t1)
            nc.vector.scalar_tensor_tensor(
                zc3[:, ts_, :], pos3[:, ts_, 1:D], 0.5, pos3[:, ts_, 0:K63],
                op0=ALU.is_lt, op1=ALU.mult,
            )
            nc.gpsimd.tensor_tensor(
                m1_3[:, ts_, :], zc3[:, ts_, :], zm1_3[:, ts_, :], op=ALU.mult)
            nc.gpsimd.tensor_tensor(
                m2_3[:, ts_, :], zc3[:, ts_, :], sdf3[:, ts_, 0:K63], op=ALU.mult)
            nc.gpsimd.tensor_tensor(
                m3_3[:, ts_, :], zc3[:, ts_, :], sdf3[:, ts_, 1:D], op=ALU.mult)

        # reductions + finals (DVE); occ ships before the s1/s2 reductions
        nc.vector.tensor_reduce(
            d1r[:], m1_3, axis=mybir.AxisListType.X, op=ALU.min)
        occ_sb = post.tile([128, NT], F32)
        nc.vector.tensor_scalar(occ_sb[:], d1r[:], -50.0, None, op0=ALU.is_le)
        nc.sync.dma_start(occ_o[:], occ_sb[:])
        d1 = post.tile([128, NT], F32)
        nc.vector.tensor_scalar(d1[:], d1r[:], 100.0, None, op0=ALU.add)
        nc.vector.tensor_reduce(
            s1[:], m2_3, axis=mybir.AxisListType.X, op=ALU.add)
        nc.vector.tensor_reduce(
            s2[:], m3_3, axis=mybir.AxisListType.X, op=ALU.add)
        den = post.tile([128, NT], F32)
        nc.vector.scalar_tensor_tensor(
            den[:], s2[:], 1e-6, s1[:], op0=ALU.subtract, op1=ALU.subtract
        )
        rec = post.tile([128, NT], F32)
        nc.vector.reciprocal(rec[:], den[:])
        interp = post.tile([128, NT], F32)
        nc.vector.scalar_tensor_tensor(
            interp[:], rec[:], t_lam, s1[:], op0=ALU.mult, op1=ALU.mult)
        res = post.tile([128, NT], F32)
        nc.vector.tensor_tensor(res[:], d1[:], interp[:], op=ALU.subtract)
        dp_sb = post.tile([128, NT], F32)
        nc.vector.tensor_tensor(dp_sb[:], occ_sb[:], res[:], op=ALU.mult)
        nc.sync.dma_start(dp_o[:], dp_sb[:])

    nc.finalize()
    return nc


def host_prep(z_shape, z_extr, W1, b1, W2, b2):
    """Per-core input maps. All small math mirrors the reference in
    float64 (deviations ~1e-7, far inside the sdf sign margins)."""
    f32 = np.float32
    z_shape = np.asarray(z_shape, f32)
    z_extr = np.asarray(z_extr, f32)
    W1 = np.asarray(W1, f32)
    b1 = np.asarray(b1, f32)
    W2 = np.asarray(W2, f32)
    b2 = np.asarray(b2, f32)

    f = 70.0 * (IMG / 64.0)
    cc = IMG / 2.0 - 0.5
    Km = np.array([[f, 0, cc], [0, f, cc], [0, 0, 1]], np.float64)
    K_inv = np.linalg.inv(Km)
    t = np.array([0.0, 0.0, 2.5])

    # mirror the reference's f32 double-reciprocal
    s_obj32 = (1.0 / z_extr[:, 0]).astype(f32)
    s_inv32 = (1.0 / s_obj32).astype(f32)
    s_obj = s_obj32.astype(np.float64)
    s_inv = s_inv32.astype(np.float64)
    t_obj = z_extr[:, 1:4].astype(np.float64)
    alpha = z_extr[:, 4].astype(np.float64)

    a = np.pi * alpha
    ca, sa = np.cos(a), np.sin(a)
    R_obj = np.zeros((BS, 3, 3))
    R_obj[:, 0, 0] = ca
    R_obj[:, 0, 1] = -sa
    R_obj[:, 1, 0] = sa
    R_obj[:, 1, 1] = ca
    R_obj[:, 2, 2] = 1.0

    corners = np.array(
        [[1, 1, 1], [1, 1, -1], [1, -1, 1], [1, -1, -1],
         [-1, 1, 1], [-1, 1, -1], [-1, -1, 1], [-1, -1, -1], [0, 0, 0]],
        np.float64,
    )
    R_obj_inv = np.linalg.inv(R_obj)
    # z-component of K @ (R_t^-1 (R_obj_inv (s_inv * corner) + t_obj) + t)
    zc = np.einsum("bij,aj->bai", R_obj_inv, corners)[:, :, 2] * s_inv[:, None]
    bb_depth = zc.mean(axis=1) + t_obj[:, 2] + 2.5      # (BS,)

    zs = np.linspace(-1.0, 1.0, D)
    depth_bd = (zs[None, :] * s_inv[:, None] + bb_depth[:, None]).astype(f32)

    M = s_obj[:, None, None] * R_obj
    c_b = np.einsum("bij,bj->bi", M, -(t[None, :] + t_obj))
    V = np.einsum("bij,ih->bjh", M, W1[:3].astype(np.float64))   # (BS,3,H)
    e = (
        np.einsum("bi,ih->bh", c_b, W1[:3].astype(np.float64))
        + z_shape.astype(np.float64) @ W1[3:].astype(np.float64)
        + b1.astype(np.float64)
    )
    e32 = e.astype(f32)
    s0 = (e32.astype(np.float64) @ W2.astype(np.float64) + b2.astype(np.float64))
    s0 = s0.astype(f32)                                  # (BS,1)

    xs = np.linspace(0.0, IMG - 1.0, IMG)
    Xg, Yg = np.meshgrid(xs, xs)
    p3 = np.stack([Xg.reshape(-1), Yg.reshape(-1), np.ones(PIX)], -1)
    q = p3 @ K_inv.T                                     # (PIX, 3)

    w2win_host = np.zeros((HID, 127), f32)
    w2win_host[:, 63] = W2[:, 0]
    in_maps = []
    for c in range(NCORES):
        b, half = c // 2, c % 2
        qs_c = q[half * PPC:(half + 1) * PPC].T.astype(f32)        # (3, PPC)
        vb_c = V[b].astype(f32)                                     # (3, H)
        qsvb_c = np.concatenate([qs_c, vb_c], axis=1)               # (3, PPC+H)
        con = np.zeros((128, 70), f32)
        con[:, C_DEPTH:C_DEPTH + 64] = depth_bd[b][None, :]
        con[:, C_NEGE] = -e32[b]
        con[:, C_EPOS] = e32[b]
        con[:, C_B0] = s0[b, 0]
        con[:, C_B1] = b2[0]
        con[:, C_LAM] = depth_bd[b][1] - depth_bd[b][0]
        # boundary tile (pixels 1280:1408): partitions < SPLIT-1280 were
        # computed max-form (bias s0), the rest relu-form (bias b2)
        con[:, C_BMIX] = np.where(np.arange(128) < SPLIT - 1280, s0[b, 0], b2[0])
        zrow1 = np.tile(depth_bd[b][0:K63] - 100.0, NT).astype(f32)
        in_maps.append({
            "qsvb": np.ascontiguousarray(qsvb_c),
            "w2w": w2win_host,
            "consts": con,
            "zm1": np.broadcast_to(zrow1, (128, NT * K63)).copy(),
        })
    return in_maps


def _assemble(results):
    f32 = np.float32
    dp_full = np.zeros((BS, PIX), f32)
    occ_full = np.zeros((BS, PIX), f32)
    for c in range(NCORES):
        b, half = c // 2, c % 2
        sl = slice(half * PPC, (half + 1) * PPC)
        dp_full[b, sl] = np.asarray(results[c]["dp"]).T.ravel()
        occ_full[b, sl] = np.asarray(results[c]["occ"]).T.ravel()
    return (
        dp_full.reshape(BS, IMG, IMG, 1),
        occ_full.reshape(BS, IMG, IMG, 1),
    )


def get_program():
    global _PROGRAM
    if _PROGRAM is None:
        _PROGRAM = build_program()
    return _PROGRAM


def kernel(z_shape, z_extr, W1, b1, W2, b2, **run_kwargs):
    nc = get_program()
    in_maps = host_prep(z_shape, z_extr, W1, b1, W2, b2)
    res = run_bass_kernel_spmd(nc, in_maps, core_ids=list(range(NCORES)), **run_kwargs)
    out = _assemble(res.results)
    if run_kwargs:
        return out, res
    return out


# revision 11
# speedup vs baseline: 1.0248x; 1.0248x over previous
"""Trainium2 Bass kernel for the DiffRenderer problem.

Math refactor (validated against the jax reference):
  The renderer's per-point MLP input collapses to
      a[b, pix, d, h] = depth[b, d] * g[b, pix, h] + e[b, h]
  with
      g[b] = Q @ V_b,  V_b = M_b^T @ W1[:3],  M_b = s_obj * R_obj
      e[b] = c_b @ W1[:3] + z_shape[b] @ W1[3:] + b1
      depth[b, d] = zs[d] * s_inv[b] + bb_depth[b]
  Layer 1 + relu:  h' = max(depth*g, -e)   (== relu(a) - e)
  Layer 2:         s  = W2 . h' + (W2 . e + b2)   (bias folded into tanh)
  sdf = tanh(s); then the zero-crossing depth extraction.

Sharding: 8 cores = 4 batches x 2 pixel-halves (2048 pixels/core, 64 depths).

v2 schedule (vs baseline):
  - all small constants packed into one [128,69] DMA; qs|vb packed [3,2176];
    input DMAs split sync queue (qsvb, consts) / gpsimd queue (w2w, zm1)
  - 6 junk warm-up matmuls on a memset tile while waiting for the qsvb DMA
    (HAM un-throttles the PE clock ~3.4us after activity starts)
  - d-loop h' split across THREE engines: DVE cols 0:1216 (mult+max),
    GPSIMD 1216:1536 (mult+max), ACT 1536:2048 (relu w/ scale+bias);
    one shared h' tile so layer-2 runs as 4 aligned 512-col matmuls
  - tail: per-chunk copy(ACT)->transpose(PE)->tanh(ACT, psum src)->
    pos(DVE); zc/m1/m3 on GPSIMD, m2+reduces+finals on DVE; occ DMA
    issued before the s1/s2 reductions
Precision: float32r (FP22 in the PE) for g and layer-2, fp32 elsewhere;
  bf16 was measured to flip sdf signs in the randn-weight regime and is
  deliberately NOT used.
"""

import os
import sys

import numpy as np

for _p in ("/opt/trn_rl_repo", "/root/.axon_site/_ro/trn_rl_repo"):
    if os.path.isdir(_p) and _p not in sys.path:
        sys.path.append(_p)

from contextlib import ExitStack

from concourse import bacc, bass, masks, mybir, tile
from concourse.bass_utils import run_bass_kernel_spmd

F32 = mybir.dt.float32
F32R = mybir.dt.float32r
ALU = mybir.AluOpType
ACTF = mybir.ActivationFunctionType

IMG = 64
D = 64
HID = 128
BS = 4
NCORES = 8
PIX = IMG * IMG          # 4096 pixels per batch
PPC = PIX // 2           # 2048 pixels per core
NT = PPC // 128          # 16 pixel tiles per core
K63 = 63                 # depth pairs per tile

# engine split of the 2048 h' columns per depth. The boundary does NOT
# need 128-alignment: the boundary tile (tile 10) gets a host-packed
# per-partition mixed tanh bias. Balanced from measured engine rates
# (DVE 179+0.59/col, ACT 149+1.21/col).
SPLIT = 1360             # DVE: 0:1360 (~981 ns/depth) max-form
                         # ACT: 1360:2048 (~982 ns/depth) relu-form
NCH = 4                  # psum chunks of 512

# consts packing: [128, 70] = depth(0:64) | nege(64) | epos(65) | bias0(66)
#                 | bias1(67) | lam(68) | bias_mix(69)
C_DEPTH, C_NEGE, C_EPOS, C_B0, C_B1, C_LAM, C_BMIX = 0, 64, 65, 66, 67, 68, 69

_PROGRAM = None


def build_program():
    nc = bacc.Bacc(None, target_bir_lowering=False)
    qsvb = nc.declare_dram_parameter("qsvb", [3, PPC + HID], F32R, isOutput=False)
    w2w = nc.declare_dram_parameter("w2w", [HID, 127], F32R, isOutput=False)
    consts = nc.declare_dram_parameter("consts", [128, 70], F32, isOutput=False)
    zm1 = nc.declare_dram_parameter("zm1", [128, NT * K63], F32, isOutput=False)
    dp_o = nc.declare_dram_parameter("dp", [128, NT], F32, isOutput=True)
    occ_o = nc.declare_dram_parameter("occ", [128, NT], F32, isOutput=True)

    with tile.TileContext(nc) as tc, ExitStack() as ctx:
        const = ctx.enter_context(tc.tile_pool(name="const", bufs=1))
        gpool = ctx.enter_context(tc.tile_pool(name="g", bufs=1))
        hpool = ctx.enter_context(tc.tile_pool(name="hp", bufs=3))
        spool = ctx.enter_context(tc.tile_pool(name="s", bufs=1))
        post = ctx.enter_context(tc.tile_pool(name="post", bufs=1))
        pss = ctx.enter_context(tc.tile_pool(name="pss", bufs=1, space="PSUM"))
        pst = ctx.enter_context(tc.tile_pool(name="pst", bufs=1, space="PSUM"))

        # ---- input DMAs: critical ones on the sync queue, rest on gpsimd
        t_consts = const.tile([128, 70], F32, name="t_consts", tag="t_consts")
        nc.sync.dma_start(t_consts[:], consts[:])
        t_qsvb = const.tile([3, PPC + HID], F32R, name="t_qsvb", tag="t_qsvb")
        nc.sync.dma_start(t_qsvb[:], qsvb[:])
        w2win = const.tile([HID, 127], F32R, name="t_w2w", tag="t_w2w")
        nc.gpsimd.dma_start(w2win[:], w2w[:])
        t_zm1 = const.tile([128, NT * K63], F32, name="t_zm1", tag="t_zm1")
        nc.gpsimd.dma_start(t_zm1[:], zm1[:])

        t_depth = t_consts[:, C_DEPTH:C_DEPTH + 64]
        t_nege = t_consts[:, C_NEGE:C_NEGE + 1]
        t_epos = t_consts[:, C_EPOS:C_EPOS + 1]
        t_bias0 = t_consts[:, C_B0:C_B0 + 1]
        t_bias1 = t_consts[:, C_B1:C_B1 + 1]
        t_lam = t_consts[:, C_LAM:C_LAM + 1]
        t_bmix = t_consts[:, C_BMIX:C_BMIX + 1]

        ident = const.tile([64, 64], F32)
        masks.make_identity(nc, ident[:])

        # ---- PE warm-up: junk matmuls on a DVE-zeroed tile while the
        # input DMAs are in flight (start=True on the real g matmul
        # discards them). fp32 4-pass mode is fine - busy is busy.
        warm = const.tile([128, 128], F32, name="warm", tag="warm")
        nc.vector.memset(warm[:], 0.0)
        pg_w = [
            pss.tile([128, 128], F32, name=f"pgw{k}", tag=f"pss{k}")
            for k in range(2)
        ]
        for j in range(8):
            nc.tensor.matmul(
                pg_w[j % 2][:], warm[:], warm[:],
                start=True, stop=True, skip_group_check=True,
            )

        # ---- g = Q @ V  (float32r matmul; one-time) ----
        g_sb = gpool.tile([HID, PPC], F32)
        t_vb = t_qsvb[:, PPC:PPC + HID]
        for k in range(NCH):
            pg = pss.tile([HID, 512], F32, name=f"pg{k}", tag=f"pss{k}")
            nc.tensor.matmul(
                pg[:], t_vb, t_qsvb[:, k * 512:(k + 1) * 512],
                start=True, stop=True, skip_group_check=True,
            )
            dst = g_sb[:, k * 512:(k + 1) * 512]
            if k % 2 == 0:
                nc.vector.tensor_copy(dst, pg[:])
            else:
                nc.scalar.copy(dst, pg[:])

        # ---- main d-loop ----
        ps_chunks = [
            pss.tile([D, 512], F32, name=f"ps_chunk{k}", tag=f"pss{k}")
            for k in range(NCH)
        ]
        for d in range(D):
            hp = hpool.tile([HID, PPC], F32R, tag="hp")
            dd = t_depth[:, d:d + 1]
            if d == 0:
                # split along the g-copy chunks so work starts before the
                # last g copy lands
                for o0, o1 in ((0, 512), (512, 1024), (1024, SPLIT)):
                    nc.vector.tensor_scalar(
                        hp[:, o0:o1], g_sb[:, o0:o1], dd, t_nege,
                        op0=ALU.mult, op1=ALU.max,
                    )
            else:
                nc.vector.tensor_scalar(
                    hp[:, 0:SPLIT], g_sb[:, 0:SPLIT], dd, t_nege,
                    op0=ALU.mult, op1=ALU.max,
                )
            nc.scalar.activation(
                hp[:, SPLIT:PPC], g_sb[:, SPLIT:PPC], ACTF.Relu,
                bias=t_epos, scale=dd,
            )
            w2slice = w2win[:, 63 - d:127 - d]
            for k in range(NCH):
                nc.tensor.matmul(
                    ps_chunks[k][:], w2slice, hp[:, k * 512:(k + 1) * 512],
                    start=(d == 0), stop=(d == D - 1),
                    skip_group_check=True,
                )

        # ---- tail ----
        s_sb = spool.tile([D, PPC], F32)
        sdf = post.tile([128, NT * D], F32)
        pos = post.tile([128, NT * D], F32)
        zc = post.tile([128, NT * K63], F32)
        m1 = post.tile([128, NT * K63], F32)
        m2 = post.tile([128, NT * K63], F32)
        m3 = post.tile([128, NT * K63], F32)
        d1r = post.tile([128, NT], F32)
        s1 = post.tile([128, NT], F32)
        s2 = post.tile([128, NT], F32)
        sdf3 = sdf[:].rearrange("p (t d) -> p t d", d=D)
        pos3 = pos[:].rearrange("p (t d) -> p t d", d=D)
        zc3 = zc[:].rearrange("p (t k) -> p t k", k=K63)
        m1_3 = m1[:].rearrange("p (t k) -> p t k", k=K63)
        m2_3 = m2[:].rearrange("p (t k) -> p t k", k=K63)
        m3_3 = m3[:].rearrange("p (t k) -> p t k", k=K63)
        zm1_3 = t_zm1[:].rearrange("p (t k) -> p t k", k=K63)

        # per chunk: copy (DVE/ACT alternating) -> 4 transposes (PE) ->
        # tanh (ACT, psum src; mixed-bias boundary tile in chunk 2)
        for k in range(NCH):
            if k % 2 == 0:
                nc.vector.tensor_copy(
                    s_sb[:, k * 512:(k + 1) * 512], ps_chunks[k][:])
            else:
                nc.scalar.copy(s_sb[:, k * 512:(k + 1) * 512], ps_chunks[k][:])
            pt = pst.tile([128, 4 * D], F32, name=f"pt{k}", tag="pt", bufs=2)
            for j in range(4):
                i = 4 * k + j
                nc.tensor.transpose(
                    pt[:, j * D:(j + 1) * D],
                    s_sb[:, i * 128:(i + 1) * 128], ident[:])
            if k == 2:
                # chunk 2 holds the DVE/ACT boundary tile (tile 10):
                # tiles 8-9 bias0, tile 10 per-partition mixed, tile 11 bias1
                for lo, hi, b_ap in ((0, 2, t_bias0), (2, 3, t_bmix),
                                     (3, 4, t_bias1)):
                    nc.scalar.activation(
                        sdf[:, (8 + lo) * D:(8 + hi) * D],
                        pt[:, lo * D:hi * D], ACTF.Tanh,
                        bias=b_ap, scale=1.0,
                    )
            else:
                b_ap = t_bias0 if k < 3 else t_bias1
                nc.scalar.activation(
                    sdf[:, 4 * k * D:4 * (k + 1) * D], pt[:], ACTF.Tanh,
                    bias=b_ap, scale=1.0,
                )

        # crossing logic in two tile-groups of 8; zc/m2 + all reduces on
        # DVE, m1/m3 on GPSIMD; per-group reduces write disjoint columns
        # so each pipelines right behind its mult
        occ_sb = post.tile([128, NT], F32)
        d1 = post.tile([128, NT], F32)
        for g_, (t0, t1) in enumerate(((0, 8), (8, 16))):
            ts_ = slice(t0, t1)
            nc.vector.tensor_scalar(
                pos[:, t0 * D:t1 * D], sdf[:, t0 * D:t1 * D],
                0.0, None, op0=ALU.is_gt)
            nc.vector.scalar_tensor_tensor(
                zc3[:, ts_, :], pos3[:, ts_, 1:D], 0.5, pos3[:, ts_, 0:K63],
                op0=ALU.is_lt, op1=ALU.mult,
            )
            nc.gpsimd.tensor_tensor(
                m1_3[:, ts_, :], zc3[:, ts_, :], zm1_3[:, ts_, :], op=ALU.mult)
            nc.gpsimd.tensor_tensor(
                m3_3[:, ts_, :], zc3[:, ts_, :], sdf3[:, ts_, 1:D], op=ALU.mult)
            nc.vector.tensor_tensor(
                m2_3[:, ts_, :], zc3[:, ts_, :], sdf3[:, ts_, 0:K63], op=ALU.mult)
            nc.vector.tensor_reduce(
                d1r[:, ts_], m1_3[:, ts_, :], axis=mybir.AxisListType.X,
                op=ALU.min)
            nc.vector.tensor_reduce(
                s1[:, ts_], m2_3[:, ts_, :], axis=mybir.AxisListType.X,
                op=ALU.add)
            nc.vector.tensor_reduce(
                s2[:, ts_], m3_3[:, ts_, :], axis=mybir.AxisListType.X,
                op=ALU.add)

        nc.vector.tensor_scalar(occ_sb[:], d1r[:], -50.0, None, op0=ALU.is_le)
        nc.sync.dma_start(occ_o[:], occ_sb[:])
        nc.vector.tensor_scalar(d1[:], d1r[:], 100.0, None, op0=ALU.add)
        den = post.tile([128, NT], F32)
        nc.vector.scalar_tensor_tensor(
            den[:], s2[:], 1e-6, s1[:], op0=ALU.subtract, op1=ALU.subtract
        )
        rec = post.tile([128, NT], F32)
        nc.vector.reciprocal(rec[:], den[:])
        interp = post.tile([128, NT], F32)
        nc.vector.scalar_tensor_tensor(
            interp[:], rec[:], t_lam, s1[:], op0=ALU.mult, op1=ALU.mult)
        res = post.tile([128, NT], F32)
        nc.vector.tensor_tensor(res[:], d1[:], interp[:], op=ALU.subtract)
        dp_sb = post.tile([128, NT], F32)
        nc.vector.tensor_tensor(dp_sb[:], occ_sb[:], res[:], op=ALU.mult)
        nc.sync.dma_start(dp_o[:], dp_sb[:])

    nc.finalize()
    return nc


def host_prep(z_shape, z_extr, W1, b1, W2, b2):
    """Per-core input maps. All small math mirrors the reference in
    float64 (deviations ~1e-7, far inside the sdf sign margins)."""
    f32 = np.float32
    z_shape = np.asarray(z_shape, f32)
    z_extr = np.asarray(z_extr, f32)
    W1 = np.asarray(W1, f32)
    b1 = np.asarray(b1, f32)
    W2 = np.asarray(W2, f32)
    b2 = np.asarray(b2, f32)

    f = 70.0 * (IMG / 64.0)
    cc = IMG / 2.0 - 0.5
    Km = np.array([[f, 0, cc], [0, f, cc], [0, 0, 1]], np.float64)
    K_inv = np.linalg.inv(Km)
    t = np.array([0.0, 0.0, 2.5])

    # mirror the reference's f32 double-reciprocal
    s_obj32 = (1.0 / z_extr[:, 0]).astype(f32)
    s_inv32 = (1.0 / s_obj32).astype(f32)
    s_obj = s_obj32.astype(np.float64)
    s_inv = s_inv32.astype(np.float64)
    t_obj = z_extr[:, 1:4].astype(np.float64)
    alpha = z_extr[:, 4].astype(np.float64)

    a = np.pi * alpha
    ca, sa = np.cos(a), np.sin(a)
    R_obj = np.zeros((BS, 3, 3))
    R_obj[:, 0, 0] = ca
    R_obj[:, 0, 1] = -sa
    R_obj[:, 1, 0] = sa
    R_obj[:, 1, 1] = ca
    R_obj[:, 2, 2] = 1.0

    corners = np.array(
        [[1, 1, 1], [1, 1, -1], [1, -1, 1], [1, -1, -1],
         [-1, 1, 1], [-1, 1, -1], [-1, -1, 1], [-1, -1, -1], [0, 0, 0]],
        np.float64,
    )
    R_obj_inv = np.linalg.inv(R_obj)
    # z-component of K @ (R_t^-1 (R_obj_inv (s_inv * corner) + t_obj) + t)
    zc = np.einsum("bij,aj->bai", R_obj_inv, corners)[:, :, 2] * s_inv[:, None]
    bb_depth = zc.mean(axis=1) + t_obj[:, 2] + 2.5      # (BS,)

    zs = np.linspace(-1.0, 1.0, D)
    depth_bd = (zs[None, :] * s_inv[:, None] + bb_depth[:, None]).astype(f32)

    M = s_obj[:, None, None] * R_obj
    c_b = np.einsum("bij,bj->bi", M, -(t[None, :] + t_obj))
    V = np.einsum("bij,ih->bjh", M, W1[:3].astype(np.float64))   # (BS,3,H)
    e = (
        np.einsum("bi,ih->bh", c_b, W1[:3].astype(np.float64))
        + z_shape.astype(np.float64) @ W1[3:].astype(np.float64)
        + b1.astype(np.float64)
    )
    e32 = e.astype(f32)
    s0 = (e32.astype(np.float64) @ W2.astype(np.float64) + b2.astype(np.float64))
    s0 = s0.astype(f32)                                  # (BS,1)

    xs = np.linspace(0.0, IMG - 1.0, IMG)
    Xg, Yg = np.meshgrid(xs, xs)
    p3 = np.stack([Xg.reshape(-1), Yg.reshape(-1), np.ones(PIX)], -1)
    q = p3 @ K_inv.T                                     # (PIX, 3)

    w2win_host = np.zeros((HID, 127), f32)
    w2win_host[:, 63] = W2[:, 0]
    in_maps = []
    for c in range(NCORES):
        b, half = c // 2, c % 2
        qs_c = q[half * PPC:(half + 1) * PPC].T.astype(f32)        # (3, PPC)
        vb_c = V[b].astype(f32)                                     # (3, H)
        qsvb_c = np.concatenate([qs_c, vb_c], axis=1)               # (3, PPC+H)
        con = np.zeros((128, 70), f32)
        con[:, C_DEPTH:C_DEPTH + 64] = depth_bd[b][None, :]
        con[:, C_NEGE] = -e32[b]
        con[:, C_EPOS] = e32[b]
        con[:, C_B0] = s0[b, 0]
        con[:, C_B1] = b2[0]
        con[:, C_LAM] = depth_bd[b][1] - depth_bd[b][0]
        # boundary tile (pixels 1280:1408): partitions < SPLIT-1280 were
        # computed max-form (bias s0), the rest relu-form (bias b2)
        con[:, C_BMIX] = np.where(np.arange(128) < SPLIT - 1280, s0[b, 0], b2[0])
        zrow1 = np.tile(depth_bd[b][0:K63] - 100.0, NT).astype(f32)
        in_maps.append({
            "qsvb": np.ascontiguousarray(qsvb_c),
            "w2w": w2win_host,
            "consts": con,
            "zm1": np.broadcast_to(zrow1, (128, NT * K63)).copy(),
        })
    return in_maps


def _assemble(results):
    f32 = np.float32
    dp_full = np.zeros((BS, PIX), f32)
    occ_full = np.zeros((BS, PIX), f32)
    for c in range(NCORES):
        b, half = c // 2, c % 2
        sl = slice(half * PPC, (half + 1) * PPC)
        dp_full[b, sl] = np.asarray(results[c]["dp"]).T.ravel()
        occ_full[b, sl] = np.asarray(results[c]["occ"]).T.ravel()
    return (
        dp_full.reshape(BS, IMG, IMG, 1),
        occ_full.reshape(BS, IMG, IMG, 1),
    )


def get_program():
    global _PROGRAM
    if _PROGRAM is None:
        _PROGRAM = build_program()
    return _PROGRAM


def kernel(z_shape, z_extr, W1, b1, W2, b2, **run_kwargs):
    nc = get_program()
    in_maps = host_prep(z_shape, z_extr, W1, b1, W2, b2)
    res = run_bass_kernel_spmd(nc, in_maps, core_ids=list(range(NCORES)), **run_kwargs)
    out = _assemble(res.results)
    if run_kwargs:
        return out, res
    return out


# revision 13
# speedup vs baseline: 1.0400x; 1.0149x over previous
"""Trainium2 Bass kernel for the DiffRenderer problem.

Math refactor (validated against the jax reference):
  The renderer's per-point MLP input collapses to
      a[b, pix, d, h] = depth[b, d] * g[b, pix, h] + e[b, h]
  with
      g[b] = Q @ V_b,  V_b = M_b^T @ W1[:3],  M_b = s_obj * R_obj
      e[b] = c_b @ W1[:3] + z_shape[b] @ W1[3:] + b1
      depth[b, d] = zs[d] * s_inv[b] + bb_depth[b]
  Layer 1 + relu:  h' = max(depth*g, -e)   (== relu(a) - e)
  Layer 2:         s  = W2 . h' + (W2 . e + b2)   (bias folded into tanh)
  sdf = tanh(s); then the zero-crossing depth extraction.

Sharding: 8 cores = 4 batches x 2 pixel-halves (2048 pixels/core, 64 depths).

v2 schedule (vs baseline):
  - all small constants packed into one [128,69] DMA; qs|vb packed [3,2176];
    input DMAs split sync queue (qsvb, consts) / gpsimd queue (w2w, zm1)
  - 6 junk warm-up matmuls on a memset tile while waiting for the qsvb DMA
    (HAM un-throttles the PE clock ~3.4us after activity starts)
  - d-loop h' split across THREE engines: DVE cols 0:1216 (mult+max),
    GPSIMD 1216:1536 (mult+max), ACT 1536:2048 (relu w/ scale+bias);
    one shared h' tile so layer-2 runs as 4 aligned 512-col matmuls
  - tail: per-chunk copy(ACT)->transpose(PE)->tanh(ACT, psum src)->
    pos(DVE); zc/m1/m3 on GPSIMD, m2+reduces+finals on DVE; occ DMA
    issued before the s1/s2 reductions
Precision: float32r (FP22 in the PE) for g and layer-2, fp32 elsewhere;
  bf16 was measured to flip sdf signs in the randn-weight regime and is
  deliberately NOT used.
"""

import os
import sys

import numpy as np

for _p in ("/opt/trn_rl_repo", "/root/.axon_site/_ro/trn_rl_repo"):
    if os.path.isdir(_p) and _p not in sys.path:
        sys.path.append(_p)

from contextlib import ExitStack

from concourse import bacc, bass, masks, mybir, tile
from concourse.bass_utils import run_bass_kernel_spmd

F32 = mybir.dt.float32
F32R = mybir.dt.float32r
ALU = mybir.AluOpType
ACTF = mybir.ActivationFunctionType

IMG = 64
D = 64
HID = 128
BS = 4
NCORES = 8
PIX = IMG * IMG          # 4096 pixels per batch
PPC = PIX // 2           # 2048 pixels per core
NT = PPC // 128          # 16 pixel tiles per core
K63 = 63                 # depth pairs per tile

# engine split of the 2048 h' columns per depth. The boundary does NOT
# need 128-alignment: the boundary tile (tile 10) gets a host-packed
# per-partition mixed tanh bias. Balanced from measured engine rates
# (DVE 179+0.59/col, ACT 149+1.21/col).
SPLIT = 1360             # DVE: 0:1360 (~981 ns/depth) max-form
                         # ACT: 1360:2048 (~982 ns/depth) relu-form
NCH = 4                  # psum chunks of 512

# consts packing: [128, 70] = depth(0:64) | nege(64) | epos(65) | bias0(66)
#                 | bias1(67) | lam(68) | bias_mix(69)
C_DEPTH, C_NEGE, C_EPOS, C_B0, C_B1, C_LAM, C_BMIX = 0, 64, 65, 66, 67, 68, 69

_PROGRAM = None


def build_program():
    nc = bacc.Bacc(None, target_bir_lowering=False)
    qsvb = nc.declare_dram_parameter("qsvb", [3, PPC + HID], F32R, isOutput=False)
    w2w = nc.declare_dram_parameter("w2w", [HID, 127], F32R, isOutput=False)
    consts = nc.declare_dram_parameter("consts", [128, 70], F32, isOutput=False)
    zm1 = nc.declare_dram_parameter("zm1", [128, NT * K63], F32, isOutput=False)
    dp_o = nc.declare_dram_parameter("dp", [128, NT], F32, isOutput=True)
    occ_o = nc.declare_dram_parameter("occ", [128, NT], F32, isOutput=True)

    with tile.TileContext(nc) as tc, ExitStack() as ctx:
        const = ctx.enter_context(tc.tile_pool(name="const", bufs=1))
        gpool = ctx.enter_context(tc.tile_pool(name="g", bufs=1))
        hpool = ctx.enter_context(tc.tile_pool(name="hp", bufs=3))
        spool = ctx.enter_context(tc.tile_pool(name="s", bufs=1))
        post = ctx.enter_context(tc.tile_pool(name="post", bufs=1))
        pss = ctx.enter_context(tc.tile_pool(name="pss", bufs=1, space="PSUM"))
        pst = ctx.enter_context(tc.tile_pool(name="pst", bufs=1, space="PSUM"))

        # ---- input DMAs: critical ones on the sync queue, rest on gpsimd
        t_consts = const.tile([128, 70], F32, name="t_consts", tag="t_consts")
        nc.sync.dma_start(t_consts[:], consts[:])
        t_qsvb = const.tile([3, PPC + HID], F32R, name="t_qsvb", tag="t_qsvb")
        nc.sync.dma_start(t_qsvb[:], qsvb[:])
        w2win = const.tile([HID, 127], F32R, name="t_w2w", tag="t_w2w")
        nc.gpsimd.dma_start(w2win[:], w2w[:])
        t_zm1 = const.tile([128, NT * K63], F32, name="t_zm1", tag="t_zm1")
        nc.gpsimd.dma_start(t_zm1[:], zm1[:])

        t_depth = t_consts[:, C_DEPTH:C_DEPTH + 64]
        t_nege = t_consts[:, C_NEGE:C_NEGE + 1]
        t_epos = t_consts[:, C_EPOS:C_EPOS + 1]
        t_bias0 = t_consts[:, C_B0:C_B0 + 1]
        t_bias1 = t_consts[:, C_B1:C_B1 + 1]
        t_lam = t_consts[:, C_LAM:C_LAM + 1]
        t_bmix = t_consts[:, C_BMIX:C_BMIX + 1]

        ident = const.tile([64, 64], F32)
        masks.make_identity(nc, ident[:])

        # ---- g = Q @ V  (float32r matmul; one-time) ----
        g_sb = gpool.tile([HID, PPC], F32)
        t_vb = t_qsvb[:, PPC:PPC + HID]
        for k in range(NCH):
            pg = pss.tile([HID, 512], F32, name=f"pg{k}", tag=f"pss{k}")
            nc.tensor.matmul(
                pg[:], t_vb, t_qsvb[:, k * 512:(k + 1) * 512],
                start=True, stop=True, skip_group_check=True,
            )
            dst = g_sb[:, k * 512:(k + 1) * 512]
            if k % 2 == 0:
                nc.vector.tensor_copy(dst, pg[:])
            else:
                nc.scalar.copy(dst, pg[:])

        # ---- main d-loop ----
        ps_chunks = [
            pss.tile([D, 512], F32, name=f"ps_chunk{k}", tag=f"pss{k}")
            for k in range(NCH)
        ]
        for d in range(D):
            hp = hpool.tile([HID, PPC], F32R, tag="hp")
            dd = t_depth[:, d:d + 1]
            if d == 0:
                # split along the g-copy chunks so work starts before the
                # last g copy lands
                for o0, o1 in ((0, 512), (512, 1024), (1024, SPLIT)):
                    nc.vector.tensor_scalar(
                        hp[:, o0:o1], g_sb[:, o0:o1], dd, t_nege,
                        op0=ALU.mult, op1=ALU.max,
                    )
            else:
                nc.vector.tensor_scalar(
                    hp[:, 0:SPLIT], g_sb[:, 0:SPLIT], dd, t_nege,
                    op0=ALU.mult, op1=ALU.max,
                )
            nc.scalar.activation(
                hp[:, SPLIT:PPC], g_sb[:, SPLIT:PPC], ACTF.Relu,
                bias=t_epos, scale=dd,
            )
            w2slice = w2win[:, 63 - d:127 - d]
            for k in range(NCH):
                nc.tensor.matmul(
                    ps_chunks[k][:], w2slice, hp[:, k * 512:(k + 1) * 512],
                    start=(d == 0), stop=(d == D - 1),
                    skip_group_check=True,
                )

        # ---- tail ----
        s_sb = spool.tile([D, PPC], F32)
        sdf = post.tile([128, NT * D], F32)
        pos = post.tile([128, NT * D], F32)
        zc = post.tile([128, NT * K63], F32)
        m1 = post.tile([128, NT * K63], F32)
        m2 = post.tile([128, NT * K63], F32)
        m3 = post.tile([128, NT * K63], F32)
        d1r = post.tile([128, NT], F32)
        s1 = post.tile([128, NT], F32)
        s2 = post.tile([128, NT], F32)
        sdf3 = sdf[:].rearrange("p (t d) -> p t d", d=D)
        pos3 = pos[:].rearrange("p (t d) -> p t d", d=D)
        zc3 = zc[:].rearrange("p (t k) -> p t k", k=K63)
        m1_3 = m1[:].rearrange("p (t k) -> p t k", k=K63)
        m2_3 = m2[:].rearrange("p (t k) -> p t k", k=K63)
        m3_3 = m3[:].rearrange("p (t k) -> p t k", k=K63)
        zm1_3 = t_zm1[:].rearrange("p (t k) -> p t k", k=K63)

        # per chunk: copy (DVE / GPSIMD) -> 4 transposes (PE) -> tanh
        # (ACT, psum src; mixed-bias boundary tile in chunk 2)
        for k in range(NCH):
            if k % 2 == 0:
                nc.vector.tensor_copy(
                    s_sb[:, k * 512:(k + 1) * 512], ps_chunks[k][:])
            else:
                nc.scalar.copy(s_sb[:, k * 512:(k + 1) * 512], ps_chunks[k][:])
            pt = pst.tile([128, 4 * D], F32, name=f"pt{k}", tag="pt", bufs=2)
            for j in range(4):
                i = 4 * k + j
                nc.tensor.transpose(
                    pt[:, j * D:(j + 1) * D],
                    s_sb[:, i * 128:(i + 1) * 128], ident[:])
            if k == 2:
                # chunk 2 holds the DVE/ACT boundary tile (tile 10):
                # tiles 8-9 bias0, tile 10 per-partition mixed, tile 11 bias1
                for lo, hi, b_ap in ((0, 2, t_bias0), (2, 3, t_bmix),
                                     (3, 4, t_bias1)):
                    nc.scalar.activation(
                        sdf[:, (8 + lo) * D:(8 + hi) * D],
                        pt[:, lo * D:hi * D], ACTF.Tanh,
                        bias=b_ap, scale=1.0,
                    )
            else:
                b_ap = t_bias0 if k < 3 else t_bias1
                nc.scalar.activation(
                    sdf[:, 4 * k * D:4 * (k + 1) * D], pt[:], ACTF.Tanh,
                    bias=b_ap, scale=1.0,
                )

        # crossing logic in two tile-groups of 8. Mults split so group 0
        # mostly feeds DVE (whose reduce queue is the critical path) and
        # group 1 mostly runs on GPSIMD while DVE reduces group 0.
        g0 = slice(0, 8)
        g1 = slice(8, 16)
        occ_sb = post.tile([128, NT], F32)
        # pos + zc per group on DVE
        nc.vector.tensor_scalar(
            pos[:, 0:8 * D], sdf[:, 0:8 * D], 0.0, None, op0=ALU.is_gt)
        nc.vector.scalar_tensor_tensor(
            zc3[:, g0, :], pos3[:, g0, 1:D], 0.5, pos3[:, g0, 0:K63],
            op0=ALU.is_lt, op1=ALU.mult)
        nc.vector.tensor_tensor(
            m1_3[:, g0, :], zc3[:, g0, :], zm1_3[:, g0, :], op=ALU.mult)
        nc.vector.tensor_tensor(
            m2_3[:, g0, :], zc3[:, g0, :], sdf3[:, g0, 0:K63], op=ALU.mult)
        nc.gpsimd.tensor_tensor(
            m3_3[:, g0, :], zc3[:, g0, :], sdf3[:, g0, 1:D], op=ALU.mult)
        nc.vector.tensor_scalar(
            pos[:, 8 * D:16 * D], sdf[:, 8 * D:16 * D], 0.0, None,
            op0=ALU.is_gt)
        nc.vector.scalar_tensor_tensor(
            zc3[:, g1, :], pos3[:, g1, 1:D], 0.5, pos3[:, g1, 0:K63],
            op0=ALU.is_lt, op1=ALU.mult)
        nc.gpsimd.tensor_tensor(
            m1_3[:, g1, :], zc3[:, g1, :], zm1_3[:, g1, :], op=ALU.mult)
        nc.gpsimd.tensor_tensor(
            m2_3[:, g1, :], zc3[:, g1, :], sdf3[:, g1, 0:K63], op=ALU.mult)
        nc.gpsimd.tensor_tensor(
            m3_3[:, g1, :], zc3[:, g1, :], sdf3[:, g1, 1:D], op=ALU.mult)
        # reduces on DVE, ordered by when their inputs land
        nc.vector.tensor_reduce(
            d1r[:, g0], m1_3[:, g0, :], axis=mybir.AxisListType.X, op=ALU.min)
        nc.vector.tensor_reduce(
            s1[:, g0], m2_3[:, g0, :], axis=mybir.AxisListType.X, op=ALU.add)
        nc.vector.tensor_reduce(
            s2[:, g0], m3_3[:, g0, :], axis=mybir.AxisListType.X, op=ALU.add)
        nc.vector.tensor_reduce(
            d1r[:, g1], m1_3[:, g1, :], axis=mybir.AxisListType.X, op=ALU.min)
        nc.vector.tensor_scalar(occ_sb[:], d1r[:], -50.0, None, op0=ALU.is_le)
        nc.sync.dma_start(occ_o[:], occ_sb[:])
        nc.vector.tensor_reduce(
            s1[:, g1], m2_3[:, g1, :], axis=mybir.AxisListType.X, op=ALU.add)
        nc.vector.tensor_reduce(
            s2[:, g1], m3_3[:, g1, :], axis=mybir.AxisListType.X, op=ALU.add)
        den = post.tile([128, NT], F32)
        nc.vector.scalar_tensor_tensor(
            den[:], s2[:], 1e-6, s1[:], op0=ALU.subtract, op1=ALU.subtract
        )
        rec = post.tile([128, NT], F32)
        nc.vector.reciprocal(rec[:], den[:])
        interp = post.tile([128, NT], F32)
        nc.vector.scalar_tensor_tensor(
            interp[:], rec[:], t_lam, s1[:], op0=ALU.mult, op1=ALU.mult)
        res = post.tile([128, NT], F32)
        nc.vector.scalar_tensor_tensor(
            res[:], d1r[:], 100.0, interp[:], op0=ALU.add, op1=ALU.subtract)
        dp_sb = post.tile([128, NT], F32)
        nc.vector.tensor_tensor(dp_sb[:], occ_sb[:], res[:], op=ALU.mult)
        nc.sync.dma_start(dp_o[:], dp_sb[:])

    nc.finalize()
    return nc


def host_prep(z_shape, z_extr, W1, b1, W2, b2):
    """Per-core input maps. All small math mirrors the reference in
    float64 (deviations ~1e-7, far inside the sdf sign margins)."""
    f32 = np.float32
    z_shape = np.asarray(z_shape, f32)
    z_extr = np.asarray(z_extr, f32)
    W1 = np.asarray(W1, f32)
    b1 = np.asarray(b1, f32)
    W2 = np.asarray(W2, f32)
    b2 = np.asarray(b2, f32)

    f = 70.0 * (IMG / 64.0)
    cc = IMG / 2.0 - 0.5
    Km = np.array([[f, 0, cc], [0, f, cc], [0, 0, 1]], np.float64)
    K_inv = np.linalg.inv(Km)
    t = np.array([0.0, 0.0, 2.5])

    # mirror the reference's f32 double-reciprocal
    s_obj32 = (1.0 / z_extr[:, 0]).astype(f32)
    s_inv32 = (1.0 / s_obj32).astype(f32)
    s_obj = s_obj32.astype(np.float64)
    s_inv = s_inv32.astype(np.float64)
    t_obj = z_extr[:, 1:4].astype(np.float64)
    alpha = z_extr[:, 4].astype(np.float64)

    a = np.pi * alpha
    ca, sa = np.cos(a), np.sin(a)
    R_obj = np.zeros((BS, 3, 3))
    R_obj[:, 0, 0] = ca
    R_obj[:, 0, 1] = -sa
    R_obj[:, 1, 0] = sa
    R_obj[:, 1, 1] = ca
    R_obj[:, 2, 2] = 1.0

    corners = np.array(
        [[1, 1, 1], [1, 1, -1], [1, -1, 1], [1, -1, -1],
         [-1, 1, 1], [-1, 1, -1], [-1, -1, 1], [-1, -1, -1], [0, 0, 0]],
        np.float64,
    )
    R_obj_inv = np.linalg.inv(R_obj)
    # z-component of K @ (R_t^-1 (R_obj_inv (s_inv * corner) + t_obj) + t)
    zc = np.einsum("bij,aj->bai", R_obj_inv, corners)[:, :, 2] * s_inv[:, None]
    bb_depth = zc.mean(axis=1) + t_obj[:, 2] + 2.5      # (BS,)

    zs = np.linspace(-1.0, 1.0, D)
    depth_bd = (zs[None, :] * s_inv[:, None] + bb_depth[:, None]).astype(f32)

    M = s_obj[:, None, None] * R_obj
    c_b = np.einsum("bij,bj->bi", M, -(t[None, :] + t_obj))
    V = np.einsum("bij,ih->bjh", M, W1[:3].astype(np.float64))   # (BS,3,H)
    e = (
        np.einsum("bi,ih->bh", c_b, W1[:3].astype(np.float64))
        + z_shape.astype(np.float64) @ W1[3:].astype(np.float64)
        + b1.astype(np.float64)
    )
    e32 = e.astype(f32)
    s0 = (e32.astype(np.float64) @ W2.astype(np.float64) + b2.astype(np.float64))
    s0 = s0.astype(f32)                                  # (BS,1)

    xs = np.linspace(0.0, IMG - 1.0, IMG)
    Xg, Yg = np.meshgrid(xs, xs)
    p3 = np.stack([Xg.reshape(-1), Yg.reshape(-1), np.ones(PIX)], -1)
    q = p3 @ K_inv.T                                     # (PIX, 3)

    w2win_host = np.zeros((HID, 127), f32)
    w2win_host[:, 63] = W2[:, 0]
    in_maps = []
    for c in range(NCORES):
        b, half = c // 2, c % 2
        qs_c = q[half * PPC:(half + 1) * PPC].T.astype(f32)        # (3, PPC)
        vb_c = V[b].astype(f32)                                     # (3, H)
        qsvb_c = np.concatenate([qs_c, vb_c], axis=1)               # (3, PPC+H)
        con = np.zeros((128, 70), f32)
        con[:, C_DEPTH:C_DEPTH + 64] = depth_bd[b][None, :]
        con[:, C_NEGE] = -e32[b]
        con[:, C_EPOS] = e32[b]
        con[:, C_B0] = s0[b, 0]
        con[:, C_B1] = b2[0]
        con[:, C_LAM] = depth_bd[b][1] - depth_bd[b][0]
        # boundary tile (pixels 1280:1408): partitions < SPLIT-1280 were
        # computed max-form (bias s0), the rest relu-form (bias b2)
        con[:, C_BMIX] = np.where(np.arange(128) < SPLIT - 1280, s0[b, 0], b2[0])
        zrow1 = np.tile(depth_bd[b][0:K63] - 100.0, NT).astype(f32)
        in_maps.append({
            "qsvb": np.ascontiguousarray(qsvb_c),
            "w2w": w2win_host,
            "consts": con,
            "zm1": np.broadcast_to(zrow1, (128, NT * K63)).copy(),
        })
    return in_maps


def _assemble(results):
    f32 = np.float32
    dp_full = np.zeros((BS, PIX), f32)
    occ_full = np.zeros((BS, PIX), f32)
    for c in range(NCORES):
        b, half = c // 2, c % 2
        sl = slice(half * PPC, (half + 1) * PPC)
        dp_full[b, sl] = np.asarray(results[c]["dp"]).T.ravel()
        occ_full[b, sl] = np.asarray(results[c]["occ"]).T.ravel()
    return (
        dp_full.reshape(BS, IMG, IMG, 1),
        occ_full.reshape(BS, IMG, IMG, 1),
    )


def get_program():
    global _PROGRAM
    if _PROGRAM is None:
        _PROGRAM = build_program()
    return _PROGRAM


def kernel(z_shape, z_extr, W1, b1, W2, b2, **run_kwargs):
    nc = get_program()
    in_maps = host_prep(z_shape, z_extr, W1, b1, W2, b2)
    res = run_bass_kernel_spmd(nc, in_maps, core_ids=list(range(NCORES)), **run_kwargs)
    out = _assemble(res.results)
    if run_kwargs:
        return out, res
    return out
